# revision 45
# baseline (speedup 1.0000x reference)
"""Trainium2 Bass kernel for nn_MultiHeadedAttention_41566693491186.

Three dual-score MHAs over the streams packed in x[:, :, 0:3, :], with shared
Wq/Wk/Wv/Wo. Data-parallel over batch B=8: one batch element per NeuronCore.

v3 design:
  - Host precomputes xT = x^T per stream and splits xT and 32*W{q,k,v} into
    fp8e4m3 (hi, lo) pairs: A ~= hi + lo with ~0.15% residual.  The nine
    input projections run as fp8 DoubleRow matmuls (2 k-tiles per pass)
    keeping hi*hi + hi*lo + lo*hi cross terms: 12 DR matmuls per [128,512]
    output tile vs 16 f32r matmuls, at near-bf16 accuracy.
  - The 32x weight scale cancels exactly: exp scale becomes 2^-14 (q and k
    both carry 32x), and the v ones-column is 32.0 so softmax denominators
    scale with the numerators.
  - All attention-side tensors (qT/kT/v spills, qcat/kcat/vext, p, OT) are
    bf16: same 1 cyc/row PE cost as f32r, half the DMA/SBUF.
  - Softmax denominators: exact DVE reciprocal + f32r K=1 broadcast matmul
    (1 cyc/row), normalize with one DVE mul into OT.
  - QK^T / PV / out-projection stay f32r-grade (bf16 inputs, f32 PSUM).

Per-core plan:
  P1  projections (interleaved with attention below):
        qT[s] = (32 x_s Wq)^T, kT[s] = (32 x_s Wk)^T  (W-stationary, [j, L])
        v[s]  =  32 x_s Wv    (x-stationary, out [L, j], interleaved with a
                               32.0 column per head for the denominators)
  P2  per (mha, head): S^T = kcat^T-chunks x qcat -> exp (ACT, scale 2^-14,
      bf16 out) -> PV accumulate o^T[d, q] + sums row -> DVE recip ->
      f32r broadcast -> DVE mul into OT (bf16).  The head inner loop is
      software-pipelined (QK(c+1) emitted before PV(c)) and attention is
      interleaved at chunk granularity with projection/output-projection
      filler so the PE never head-of-line blocks on the ACT exp.
  P3  out = OT^T @ Wo + bo  (OT-stationary, out [q, d_model]) -> DRAM.
"""

import sys

if "/opt/trn_rl_repo" not in sys.path:
    sys.path.insert(0, "/opt/trn_rl_repo")

import numpy as np

B, L, D = 8, 1024, 1024
H, DH = 16, 64
NCH = 8              # 128-sized chunks along D or L
SCALE = 0.0625 / 1024.0   # (1/sqrt(64)) * 0.5 / (32*32)
WSCL = 32.0
N_CORES = 8
# mha m reads (A, B, V) streams: q1/k1 from A, q2/k2 from B, v from V
MHA_STREAMS = ((1, 2, 0), (0, 2, 1), (0, 1, 2))

_CACHE = {}


def _split_excess_waits(nc, max_waits=1):
    """Stock neuronxcc walrus rejects instructions carrying more than
    `max_waits` semaphore waits; move excess onto same-engine NOPs."""
    import concourse.mybir as mybir

    for f in nc.m.functions:
        for bb in f.blocks:
            out = []
            changed = False
            for inst in bb.instructions:
                si = inst.sync_info
                waits = list(si.on_wait) if (si is not None and si.on_wait) else []
                if len(waits) > max_waits:
                    extra, keep = waits[:-max_waits], waits[-max_waits:]
                    k = 0
                    while extra:
                        chunk, extra = extra[:max_waits], extra[max_waits:]
                        nop = mybir.InstNoOp(
                            name=f"{inst.name}-ws{k}",
                            engine=inst.engine,
                            sync_info=mybir.SyncInfo(on_wait=chunk, on_update=[]),
                        )
                        out.append(nop)
                        k += 1
                    inst.sync_info = mybir.SyncInfo(
                        on_wait=keep,
                        on_update=list(si.on_update) if si.on_update else [],
                    )
                    changed = True
                out.append(inst)
            if changed:
                bb.instructions = out


def _interleave(*seqs):
    """Proportional merge of thunk lists, preserving within-list order."""
    items = []
    for si, seq in enumerate(seqs):
        n = len(seq)
        for i, thunk in enumerate(seq):
            items.append(((i + 0.5) / n, si, i, thunk))
    for _, _, _, t in sorted(items, key=lambda z: (z[0], z[1], z[2])):
        t()


def _build_program(repeat=1):
    import concourse.bass as bass
    import concourse.mybir as mybir
    import concourse.tile as tile

    f32 = mybir.dt.float32
    f32r = mybir.dt.float32r
    bf16 = mybir.dt.bfloat16
    f8 = mybir.dt.float8e4
    DR = mybir.MatmulPerfMode.DoubleRow
    AF = mybir.ActivationFunctionType

    nc = bass.Bass("TRN2", target_bir_lowering=False, debug=False)

    # hi/lo fp8 pairs, packed [2, D, L]: index 0 = hi, 1 = lo
    xT8 = nc.declare_dram_parameter("xT8", [3, 2, D, L], f8, isOutput=False)
    Wq8 = nc.declare_dram_parameter("Wq8", [2, D, D], f8, isOutput=False)
    Wk8 = nc.declare_dram_parameter("Wk8", [2, D, D], f8, isOutput=False)
    Wv8 = nc.declare_dram_parameter("Wv8", [2, D, D], f8, isOutput=False)
    Wo = nc.declare_dram_parameter("Wo", [D, D], bf16, isOutput=False)
    bq = nc.declare_dram_parameter("bq", [D], f32, isOutput=False)
    bk = nc.declare_dram_parameter("bk", [D], f32, isOutput=False)
    out = nc.declare_dram_parameter("out", [L, 3, D], f32, isOutput=True)

    # internal DRAM spill (bf16)
    qT_d = [nc.dram_tensor(f"qT{s}", [D, L], bf16) for s in range(3)]
    kT_d = [nc.dram_tensor(f"kT{s}", [D, L], bf16) for s in range(3)]
    # v: head h data at cols 65h..65h+64, 32.0 column at 65h+64
    v_d = [nc.dram_tensor(f"v{s}", [L, H * 65], bf16) for s in range(3)]

    with tile.TileContext(nc) as tc:
        cstack = []
        cp = tc.alloc_tile_pool(name="const", bufs=1)
        psum = tc.alloc_tile_pool(name="psum", bufs=1, space="PSUM")
        xts = tc.alloc_tile_pool(name="xts", bufs=4)
        cstack += [cp, psum, xts]

        cmisc = cp.tile([128, 96], f32, tag="cmisc", name="cmisc")
        ones64 = cmisc[:, 0:64]
        v32c = cmisc[:, 80:96]
        bq_t = cmisc[:, 64:72]
        bk_t = cmisc[:, 72:80]
        nc.gpsimd.memset(ones64, 1.0)
        nc.gpsimd.memset(v32c, WSCL)
        onesr = cp.tile([1, 64], f32r, tag="onesr", name="onesr")
        nc.vector.tensor_copy(onesr[:], ones64[0:1, :])
        ones_r = onesr
        nc.sync.dma_start(out=bq_t, in_=bq.rearrange("(c p) -> p c", p=128))
        nc.sync.dma_start(out=bk_t, in_=bk.rearrange("(c p) -> p c", p=128))

        # ---------------- xT loads (fp8 hi+lo, host pre-transposed) ---------
        xt_tiles = {}

        def load_xt(s, t_only=None):
            # [128, hilo, c, l] fp8
            if t_only in (None, 0):
                xt = xts.tile([128, 2, NCH, L], f8, tag="xts", name=f"xt{s}")
                xt_tiles[s] = xt
            xt = xt_tiles[s]
            src = xT8[s].rearrange("t (c p) l -> p t c l", p=128)
            for t in ((0, 1) if t_only is None else (t_only,)):
                nc.sync.dma_start(out=xt[:, t, 0:4, :], in_=src[:, t, 0:4, :])
                nc.scalar.dma_start(out=xt[:, t, 4:8, :],
                                    in_=src[:, t, 4:8, :])

        # ---------------- shared pools for P1/P2/P3 ----------------
        wrp = tc.alloc_tile_pool(name="wrp", bufs=1)
        stp = tc.alloc_tile_pool(name="stp", bufs=5)
        qkp = tc.alloc_tile_pool(name="qkp", bufs=3)
        ptp = tc.alloc_tile_pool(name="ptp", bufs=5)
        rbp = tc.alloc_tile_pool(name="rbp", bufs=2)
        cstack += [wrp, stp, qkp, ptp, rbp]

        def load_w8(Wsrc, w_t=None, t_only=None):
            # [128, hilo, c, d] fp8
            if w_t is None:
                w_t = wrp.tile([128, 2, NCH, D], f8, tag="W8", name="W8",
                               bufs=2)
            src = Wsrc.rearrange("t (c p) d -> p t c d", p=128)
            for t in ((0, 1) if t_only is None else (t_only,)):
                nc.sync.dma_start(out=w_t[:, t, 0:4, :], in_=src[:, t, 0:4, :])
                nc.scalar.dma_start(out=w_t[:, t, 4:8, :], in_=src[:, t, 4:8, :])
            return w_t

        def load_wo(Wsrc):
            w_t = wrp.tile([128, NCH * D], bf16, tag="Wor", name="Wor")
            d3 = w_t[:].rearrange("p (c d) -> p c d", d=D)
            s3 = Wsrc.rearrange("(c p) d -> p c d", p=128)
            nc.sync.dma_start(out=d3[:, 0:4, :], in_=s3[:, 0:4, :])
            nc.scalar.dma_start(out=d3[:, 4:8, :], in_=s3[:, 4:8, :])
            return w_t

        def dr_products(emit, w_t, xt):
            """12 DoubleRow matmuls accumulating hi*hi + hi*lo + lo*hi over
            4 chunk-pairs; emit(lhs_sel, rhs_sel, t, first, last)."""
            combos = ((0, 0), (0, 1), (1, 0))
            n = 0
            for t in range(4):
                for (wi, xi) in combos:
                    n += 1
                    emit(wi, xi, t, n == 1, n == 12)

        def proj_qk_twopass(w_t, b_t, s, outd, xt):
            # pass 1: hi*hi only (needs just the hi halves); pass 2: the two
            # cross products accumulated in PSUM then DVE-added into st
            sts = {}

            def pass1(jc, lh):
                ps = psum.tile([128, 512], f32, tag="pp", name="pp", bufs=2)
                for t in range(4):
                    nc.tensor.matmul(
                        ps[:],
                        lhsT=w_t[:, 0, 2 * t:2 * t + 2,
                                 128 * jc:128 * (jc + 1)],
                        rhs=xt[:, 0, 2 * t:2 * t + 2,
                               512 * lh:512 * (lh + 1)],
                        start=(t == 0), stop=(t == 3), perf_mode=DR)
                if lh == 0:
                    sts[jc] = stp.tile([128, L], bf16, tag="stq", name="st", bufs=9)
                nc.vector.tensor_scalar_add(
                    sts[jc][:, 512 * lh:512 * (lh + 1)], ps[:],
                    b_t[:, jc:jc + 1])

            def pass2(jc, lh):
                ps = psum.tile([128, 512], f32, tag="pp", name="pp", bufs=2)
                n = 0
                for t in range(4):
                    for (wi, xi) in ((0, 1), (1, 0)):
                        n += 1
                        nc.tensor.matmul(
                            ps[:],
                            lhsT=w_t[:, wi, 2 * t:2 * t + 2,
                                     128 * jc:128 * (jc + 1)],
                            rhs=xt[:, xi, 2 * t:2 * t + 2,
                                   512 * lh:512 * (lh + 1)],
                            start=(n == 1), stop=(n == 8), perf_mode=DR)
                sl = sts[jc][:, 512 * lh:512 * (lh + 1)]
                nc.vector.tensor_add(sl, sl, ps[:])
                if lh == 1:
                    nc.scalar.dma_start(
                        out=outd[s][128 * jc:128 * (jc + 1), :],
                        in_=sts[jc][:])

            return ([lambda jc=jc, lh=lh: pass1(jc, lh)
                     for jc in range(NCH) for lh in range(2)]
                    + [lambda jc=jc, lh=lh: pass2(jc, lh)
                       for jc in range(NCH) for lh in range(2)])

        def proj_qk_blocks(w_t, b_t, s, outd, xt):
            # out [j, L] = (32 x_s W)^T, one block per jc
            def block(jc):
                def run():
                    st = stp.tile([128, L], bf16, tag="stq", name="st", bufs=9)
                    for lh in range(2):
                        ps = psum.tile([128, 512], f32, tag="pp",
                                       name="pp", bufs=2)

                        def emit(wi, xi, t, first, last, lh=lh, ps=ps):
                            nc.tensor.matmul(
                                ps[:],
                                lhsT=w_t[:, wi, 2 * t:2 * t + 2,
                                         128 * jc:128 * (jc + 1)],
                                rhs=xt[:, xi, 2 * t:2 * t + 2,
                                       512 * lh:512 * (lh + 1)],
                                start=first, stop=last, perf_mode=DR)
                        dr_products(emit, w_t, xt)
                        nc.vector.tensor_scalar_add(
                            st[:, 512 * lh:512 * (lh + 1)], ps[:],
                            b_t[:, jc:jc + 1])
                    nc.scalar.dma_start(
                        out=outd[s][128 * jc:128 * (jc + 1), :], in_=st[:])
                return run
            return [block(jc) for jc in range(NCH)]

        def proj_v_blocks(w_t, s, xt):
            # out [L, j] with interleaved 32.0 columns, one block per lc
            def block(lc):
                def run():
                    for jh in range(2):
                        ps = psum.tile([128, 512], f32, tag="pp",
                                       name="pp", bufs=2)

                        def emit(wi, xi, t, first, last, jh=jh, ps=ps):
                            nc.tensor.matmul(
                                ps[:],
                                lhsT=xt[:, xi, 2 * t:2 * t + 2,
                                        128 * lc:128 * (lc + 1)],
                                rhs=w_t[:, wi, 2 * t:2 * t + 2,
                                        512 * jh:512 * (jh + 1)],
                                start=first, stop=last, perf_mode=DR)
                        dr_products(emit, w_t, xt)
                        vst = stp.tile([128, 8 * 65], bf16, tag="stv", name="vst")
                        r = vst[:].rearrange("p (h w) -> p h w", w=65)
                        q3 = ps[:].rearrange("p (h w) -> p h w", w=64)
                        nc.vector.tensor_copy(r[:, :, 0:64], q3)
                        nc.vector.tensor_copy(
                            r[:, :, 64:65].squeeze(2), v32c[:, 0:8])
                        nc.scalar.dma_start(
                            out=v_d[s][128 * lc:128 * (lc + 1),
                                       8 * 65 * jh:8 * 65 * (jh + 1)],
                            in_=vst[:])
                return run
            return [block(lc) for lc in range(NCH)]

        def proj_v_units(w_t, s, xt):
            # one unit per (lc, jh) half-block
            def unit(lc, jh):
                def run():
                    ps = psum.tile([128, 512], f32, tag="pp",
                                   name="pp", bufs=2)

                    def emit(wi, xi, t, first, last):
                        nc.tensor.matmul(
                            ps[:],
                            lhsT=xt[:, xi, 2 * t:2 * t + 2,
                                    128 * lc:128 * (lc + 1)],
                            rhs=w_t[:, wi, 2 * t:2 * t + 2,
                                    512 * jh:512 * (jh + 1)],
                            start=first, stop=last, perf_mode=DR)
                    dr_products(emit, w_t, xt)
                    vst = stp.tile([128, 8 * 65], bf16, tag="stv", name="vst")
                    r = vst[:].rearrange("p (h w) -> p h w", w=65)
                    q3 = ps[:].rearrange("p (h w) -> p h w", w=64)
                    nc.vector.tensor_copy(r[:, :, 0:64], q3)
                    nc.vector.tensor_copy(
                        r[:, :, 64:65].squeeze(2), v32c[:, 0:8])
                    nc.scalar.dma_start(
                        out=v_d[s][128 * lc:128 * (lc + 1),
                                   8 * 65 * jh:8 * 65 * (jh + 1)],
                        in_=vst[:])
                return run
            return [unit(lc, jh) for lc in range(NCH) for jh in range(2)]

        def attention_blocks(m, OT):
            sa, sb, sv = MHA_STREAMS[m]
            pend = {}
            pend2 = {}

            def finalize(h, oc):
                po, co = 64 * (h % 2), (h // 2) * L
                rb_r = rbp.tile([1, L], f32r, tag="rbr", name="rbr")
                with nc.allow_low_precision(reason="recip feeds f32r bcast"):
                    nc.vector.reciprocal(rb_r[0:1, :], oc[64:65, :])
                for qh in range(2):
                    rb_ps = psum.tile([64, 512], f32, tag="pp",
                                      name="rb_ps", bufs=2)
                    nc.tensor.matmul(
                        rb_ps[:], lhsT=ones_r[0:1, :],
                        rhs=rb_r[0:1, 512 * qh:512 * (qh + 1)],
                        start=True, stop=True)
                    nc.vector.tensor_mul(
                        OT[po:po + 64, co + 512 * qh:co + 512 * (qh + 1)],
                        oc[0:64, 512 * qh:512 * (qh + 1)], rb_ps[:])

            def step_pend(h, oc):
                if pend:
                    (h1, oc1), = pend.items()
                    finalize(h1, oc1)
                    pend.clear()
                if h is not None:
                    pend[h] = oc

            def head_units(h):
                """Chunk-granular thunks: [load+QK0, (QK1,PV0), ...,
                (QK7,PV6), (PV7,copies,finalize-prev)]."""
                st = {}

                def qk(c):
                    s_ps = psum.tile([128, L], f32, tag="scr", name="scr",
                                     bufs=2)
                    for qh in range(2):
                        nc.tensor.matmul(
                            s_ps[:, 512 * qh:512 * (qh + 1)],
                            lhsT=st["kcat"][:, 128 * c:128 * (c + 1)],
                            rhs=st["qcat"][:, 512 * qh:512 * (qh + 1)],
                            start=True, stop=True)
                    p_sb = ptp.tile([128, L], bf16, tag="p_sb", name="p_sb")
                    nc.scalar.activation(p_sb[:], s_ps[:], AF.Exp, scale=SCALE)
                    st[c] = p_sb

                def pv(c):
                    for qh in range(2):
                        nc.tensor.matmul(
                            st["o_ps"][qh][0:65, :],
                            lhsT=st["vext"][:, 65 * c:65 * (c + 1)],
                            rhs=st[c][:, 512 * qh:512 * (qh + 1)],
                            start=(c == 0), stop=(c == NCH - 1))
                    del st[c]

                def u_load():
                    qcat = qkp.tile([128, L], bf16, tag="qcat", name="qcat",
                                    bufs=6)
                    kcat = qkp.tile([128, L], bf16, tag="kcat", name="kcat",
                                    bufs=6)
                    nc.sync.dma_start(
                        out=qcat[0:64, :], in_=qT_d[sa][64 * h:64 * h + 64, :])
                    nc.sync.dma_start(
                        out=qcat[64:128, :], in_=qT_d[sb][64 * h:64 * h + 64, :])
                    nc.sync.dma_start(
                        out=kcat[0:64, :], in_=kT_d[sa][64 * h:64 * h + 64, :])
                    nc.sync.dma_start(
                        out=kcat[64:128, :], in_=kT_d[sb][64 * h:64 * h + 64, :])
                    vext = qkp.tile([128, NCH * 65], bf16, tag="vext",
                                    name="vext", bufs=4)
                    vsrc = v_d[sv].rearrange("(c p) w -> p c w", p=128)
                    nc.sync.dma_start(
                        out=vext[:].rearrange("p (c w) -> p c w", w=65),
                        in_=vsrc[:, :, 65 * h:65 * (h + 1)])
                    st["qcat"], st["kcat"], st["vext"] = qcat, kcat, vext
                    st["o_ps"] = [psum.tile([65, 512], f32, tag="ops",
                                            name="ops", bufs=2)
                                  for _ in range(2)]
                    qk(0)

                def u_mid(c):
                    qk(c)
                    pv(c - 1)

                def u_tail():
                    pv(NCH - 1)
                    oc = stp.tile([65, L], f32, tag="stoc", name="oc",
                                  bufs=6)
                    for qh in range(2):
                        nc.vector.tensor_copy(
                            oc[:, 512 * qh:512 * (qh + 1)],
                            st["o_ps"][qh][:])
                    step_pend(h, oc)

                return ([u_load] + [lambda c=c: u_mid(c)
                                    for c in range(1, NCH)] + [u_tail])

            def tail():
                step_pend(None, None)

            units = []
            for h in range(H):
                units += head_units(h)
            units.append(tail)
            return units

        def oproj_units(m, OT, wo_t):
            # one unit per (qc, dh) half-block; DMA issued on the dh=1 unit
            osts = {}

            def unit(qc, dh):
                def run():
                    if dh == 0:
                        osts[qc] = stp.tile([128, L], f32, tag="stoc",
                                            name="ost", bufs=6)
                    ost = osts[qc]
                    op_ps = psum.tile([128, 512], f32, tag="pp",
                                      name="pp", bufs=2)
                    for c in range(NCH):
                        nc.tensor.matmul(
                            op_ps[:],
                            lhsT=OT[:, L * c + 128 * qc:L * c + 128 * (qc + 1)],
                            rhs=wo_t[:, D * c + 512 * dh:D * c + 512 * (dh + 1)],
                            start=(c == 0), stop=(c == NCH - 1))
                    nc.vector.tensor_copy(
                        ost[:, 512 * dh:512 * (dh + 1)], op_ps[:])
                    if dh == 1:
                        nc.scalar.dma_start(
                            out=out[128 * qc:128 * (qc + 1), m, :], in_=ost[:])
                        del osts[qc]
                return run
            return [unit(qc, dh) for qc in range(NCH) for dh in range(2)]

        for _rep in range(repeat):
            # ---------------- emission schedule ----------------
            OTs = {}

            def mk_ot(m):
                OTs[m] = xts.tile([128, NCH * L], bf16, tag="xts", name="OT")

            wq_t = load_w8(Wq8, t_only=0)
            load_xt(1, t_only=0)
            load_w8(Wq8, w_t=wq_t, t_only=1)
            load_xt(1, t_only=1)
            load_xt(2)
            load_xt(0)
            for b in proj_qk_twopass(wq_t, bq_t, 1, qT_d, xt_tiles[1]):
                b()
            wk_t = load_w8(Wk8)
            for s in (2, 0):
                for b in proj_qk_blocks(wq_t, bq_t, s, qT_d, xt_tiles[s]):
                    b()
            for b in proj_qk_blocks(wk_t, bk_t, 1, kT_d, xt_tiles[1]):
                b()
            wv_t = load_w8(Wv8)
            for s in (2, 0):
                for b in proj_qk_blocks(wk_t, bk_t, s, kT_d, xt_tiles[s]):
                    b()

            for b in proj_v_blocks(wv_t, 0, xt_tiles[0]):
                b()

            # A0 || (v1, v2): xt1/xt2 and Wv still resident
            mk_ot(0)

            _interleave(attention_blocks(0, OTs[0]),
                        proj_v_units(wv_t, 1, xt_tiles[1])
                        + proj_v_units(wv_t, 2, xt_tiles[2]))

            # A1 || (load Wo, oproj 0)
            mk_ot(1)
            wo_state = {}

            def o0_blocks():
                blocks = []

                def loadwo():
                    wo_state["w"] = load_wo(Wo)
                blocks.append(loadwo)

                def get_units():
                    if "u0" not in wo_state:
                        wo_state["u0"] = oproj_units(0, OTs[0], wo_state["w"])
                    return wo_state["u0"]
                for u in range(16):
                    blocks.append(lambda u=u: get_units()[u]())
                return blocks

            _interleave(attention_blocks(1, OTs[1]), o0_blocks())

            # A2 || oproj 1
            mk_ot(2)
            o1_units = oproj_units(1, OTs[1], wo_state["w"])
            _interleave(attention_blocks(2, OTs[2]),
                        [lambda u=u: o1_units[u]() for u in range(16)])

            for u in oproj_units(2, OTs[2], wo_state["w"]):
                u()

        for p in reversed(cstack):
            p.release()

    _split_excess_waits(nc, max_waits=1)
    return nc


def get_program():
    if "nc" not in _CACHE:
        _CACHE["nc"] = _build_program()
    return _CACHE["nc"]


def _split_fp8(a, axis=0):
    """a (f32) -> (hi, lo) fp8e4m3 stacked on `axis` with hi + lo ~= a."""
    import ml_dtypes

    hi = a.astype(ml_dtypes.float8_e4m3)
    lo = (a - hi.astype(np.float32)).astype(ml_dtypes.float8_e4m3)
    return np.ascontiguousarray(np.stack([hi, lo], axis=axis))


def kernel(x, Wq, bq, Wk, bk, Wv, bv, Wo, bo):
    import ml_dtypes
    from concourse.bass_utils import run_bass_kernel_spmd

    nc = get_program()
    x = np.ascontiguousarray(np.asarray(x, dtype=np.float32))
    Wq = np.asarray(Wq, dtype=np.float32)
    Wk = np.asarray(Wk, dtype=np.float32)
    Wv = np.asarray(Wv, dtype=np.float32)
    ws = {
        "Wq8": _split_fp8(WSCL * Wq),
        "Wk8": _split_fp8(WSCL * Wk),
        "Wv8": _split_fp8(WSCL * Wv),
        "Wo": np.ascontiguousarray(np.asarray(Wo, dtype=np.float32).astype(ml_dtypes.bfloat16)),
        "bq": WSCL * np.asarray(bq, dtype=np.float32),
        "bk": WSCL * np.asarray(bk, dtype=np.float32),
    }
    bv = np.asarray(bv, dtype=np.float64)
    bo = np.asarray(bo, dtype=np.float64)
    in_maps = [
        dict(ws, xT8=_split_fp8(x[b].transpose(1, 2, 0), axis=1))
        for b in range(N_CORES)
    ]
    res = run_bass_kernel_spmd(nc, in_maps, list(range(N_CORES)))
    outp = np.stack([res.results[b]["out"] for b in range(N_CORES)], axis=0)
    # bv and bo fold into a constant output row: softmax rows sum to 1, so
    # attention(v + bv) = attention(v) + bv, and (o + bv) @ Wo + bo adds
    # (bv @ Wo + bo) to every output row.
    corr = bv @ np.asarray(Wo, dtype=np.float64) + bo
    if np.any(corr):
        outp = (outp.astype(np.float64) + corr[None, None, None, :]).astype(
            np.float32)
    return outp


# revision 46
# speedup vs baseline: 1.0034x; 1.0034x over previous
"""Trainium2 Bass kernel for nn_MultiHeadedAttention_41566693491186.

Three dual-score MHAs over the streams packed in x[:, :, 0:3, :], with shared
Wq/Wk/Wv/Wo. Data-parallel over batch B=8: one batch element per NeuronCore.

v3 design:
  - Host precomputes xT = x^T per stream and splits xT and 32*W{q,k,v} into
    fp8e4m3 (hi, lo) pairs: A ~= hi + lo with ~0.15% residual.  The nine
    input projections run as fp8 DoubleRow matmuls (2 k-tiles per pass)
    keeping hi*hi + hi*lo + lo*hi cross terms: 12 DR matmuls per [128,512]
    output tile vs 16 f32r matmuls, at near-bf16 accuracy.
  - The 32x weight scale cancels exactly: exp scale becomes 2^-14 (q and k
    both carry 32x), and the v ones-column is 32.0 so softmax denominators
    scale with the numerators.
  - All attention-side tensors (qT/kT/v spills, qcat/kcat/vext, p, OT) are
    bf16: same 1 cyc/row PE cost as f32r, half the DMA/SBUF.
  - Softmax denominators: exact DVE reciprocal + f32r K=1 broadcast matmul
    (1 cyc/row), normalize with one DVE mul into OT.
  - QK^T / PV / out-projection stay f32r-grade (bf16 inputs, f32 PSUM).

Per-core plan:
  P1  projections (interleaved with attention below):
        qT[s] = (32 x_s Wq)^T, kT[s] = (32 x_s Wk)^T  (W-stationary, [j, L])
        v[s]  =  32 x_s Wv    (x-stationary, out [L, j], interleaved with a
                               32.0 column per head for the denominators)
  P2  per (mha, head): S^T = kcat^T-chunks x qcat -> exp (ACT, scale 2^-14,
      bf16 out) -> PV accumulate o^T[d, q] + sums row -> DVE recip ->
      f32r broadcast -> DVE mul into OT (bf16).  The head inner loop is
      software-pipelined (QK(c+1) emitted before PV(c)) and attention is
      interleaved at chunk granularity with projection/output-projection
      filler so the PE never head-of-line blocks on the ACT exp.
  P3  out = OT^T @ Wo + bo  (OT-stationary, out [q, d_model]) -> DRAM.
"""

import sys

if "/opt/trn_rl_repo" not in sys.path:
    sys.path.insert(0, "/opt/trn_rl_repo")

import numpy as np

B, L, D = 8, 1024, 1024
H, DH = 16, 64
NCH = 8              # 128-sized chunks along D or L
SCALE = 0.0625 / 1024.0   # (1/sqrt(64)) * 0.5 / (32*32)
WSCL = 32.0
N_CORES = 8
# mha m reads (A, B, V) streams: q1/k1 from A, q2/k2 from B, v from V
MHA_STREAMS = ((1, 2, 0), (0, 2, 1), (0, 1, 2))

_CACHE = {}


def _split_excess_waits(nc, max_waits=1):
    """Stock neuronxcc walrus rejects instructions carrying more than
    `max_waits` semaphore waits; move excess onto same-engine NOPs."""
    import concourse.mybir as mybir

    for f in nc.m.functions:
        for bb in f.blocks:
            out = []
            changed = False
            for inst in bb.instructions:
                si = inst.sync_info
                waits = list(si.on_wait) if (si is not None and si.on_wait) else []
                if len(waits) > max_waits:
                    extra, keep = waits[:-max_waits], waits[-max_waits:]
                    k = 0
                    while extra:
                        chunk, extra = extra[:max_waits], extra[max_waits:]
                        nop = mybir.InstNoOp(
                            name=f"{inst.name}-ws{k}",
                            engine=inst.engine,
                            sync_info=mybir.SyncInfo(on_wait=chunk, on_update=[]),
                        )
                        out.append(nop)
                        k += 1
                    inst.sync_info = mybir.SyncInfo(
                        on_wait=keep,
                        on_update=list(si.on_update) if si.on_update else [],
                    )
                    changed = True
                out.append(inst)
            if changed:
                bb.instructions = out


def _interleave(*seqs):
    """Proportional merge of thunk lists, preserving within-list order.
    A list may be passed as (list, phase) to bias its positions earlier
    (phase < 0.5) or later within each merge window."""
    items = []
    for si, seq in enumerate(seqs):
        off = 0.5
        if isinstance(seq, tuple):
            seq, off = seq
        n = len(seq)
        for i, thunk in enumerate(seq):
            items.append(((i + off) / n, si, i, thunk))
    for _, _, _, t in sorted(items, key=lambda z: (z[0], z[1], z[2])):
        t()


def _build_program(repeat=1):
    import concourse.bass as bass
    import concourse.mybir as mybir
    import concourse.tile as tile

    f32 = mybir.dt.float32
    f32r = mybir.dt.float32r
    bf16 = mybir.dt.bfloat16
    f8 = mybir.dt.float8e4
    DR = mybir.MatmulPerfMode.DoubleRow
    AF = mybir.ActivationFunctionType

    nc = bass.Bass("TRN2", target_bir_lowering=False, debug=False)

    # hi/lo fp8 pairs, packed [2, D, L]: index 0 = hi, 1 = lo
    xT8 = nc.declare_dram_parameter("xT8", [3, 2, D, L], f8, isOutput=False)
    Wq8 = nc.declare_dram_parameter("Wq8", [2, D, D], f8, isOutput=False)
    Wk8 = nc.declare_dram_parameter("Wk8", [2, D, D], f8, isOutput=False)
    Wv8 = nc.declare_dram_parameter("Wv8", [2, D, D], f8, isOutput=False)
    Wo = nc.declare_dram_parameter("Wo", [D, D], bf16, isOutput=False)
    bq = nc.declare_dram_parameter("bq", [D], f32, isOutput=False)
    bk = nc.declare_dram_parameter("bk", [D], f32, isOutput=False)
    out = nc.declare_dram_parameter("out", [L, 3, D], f32, isOutput=True)

    # internal DRAM spill (bf16)
    qT_d = [nc.dram_tensor(f"qT{s}", [D, L], bf16) for s in range(3)]
    kT_d = [nc.dram_tensor(f"kT{s}", [D, L], bf16) for s in range(3)]
    # v: head h data at cols 65h..65h+64, 32.0 column at 65h+64
    v_d = [nc.dram_tensor(f"v{s}", [L, H * 65], bf16) for s in range(3)]

    with tile.TileContext(nc) as tc:
        cstack = []
        cp = tc.alloc_tile_pool(name="const", bufs=1)
        psum = tc.alloc_tile_pool(name="psum", bufs=1, space="PSUM")
        xts = tc.alloc_tile_pool(name="xts", bufs=4)
        cstack += [cp, psum, xts]

        cmisc = cp.tile([128, 96], f32, tag="cmisc", name="cmisc")
        ones64 = cmisc[:, 0:64]
        v32c = cmisc[:, 80:96]
        bq_t = cmisc[:, 64:72]
        bk_t = cmisc[:, 72:80]
        nc.gpsimd.memset(ones64, 1.0)
        nc.gpsimd.memset(v32c, WSCL)
        onesr = cp.tile([1, 64], f32r, tag="onesr", name="onesr")
        nc.vector.tensor_copy(onesr[:], ones64[0:1, :])
        ones_r = onesr
        nc.sync.dma_start(out=bq_t, in_=bq.rearrange("(c p) -> p c", p=128))
        nc.sync.dma_start(out=bk_t, in_=bk.rearrange("(c p) -> p c", p=128))

        # ---------------- xT loads (fp8 hi+lo, host pre-transposed) ---------
        xt_tiles = {}

        def load_xt(s, t_only=None):
            # [128, hilo, c, l] fp8
            if t_only in (None, 0):
                xt = xts.tile([128, 2, NCH, L], f8, tag="xts", name=f"xt{s}")
                xt_tiles[s] = xt
            xt = xt_tiles[s]
            src = xT8[s].rearrange("t (c p) l -> p t c l", p=128)
            for t in ((0, 1) if t_only is None else (t_only,)):
                nc.sync.dma_start(out=xt[:, t, 0:4, :], in_=src[:, t, 0:4, :])
                nc.scalar.dma_start(out=xt[:, t, 4:8, :],
                                    in_=src[:, t, 4:8, :])

        # ---------------- shared pools for P1/P2/P3 ----------------
        wrp = tc.alloc_tile_pool(name="wrp", bufs=1)
        stp = tc.alloc_tile_pool(name="stp", bufs=5)
        qkp = tc.alloc_tile_pool(name="qkp", bufs=3)
        ptp = tc.alloc_tile_pool(name="ptp", bufs=5)
        rbp = tc.alloc_tile_pool(name="rbp", bufs=2)
        cstack += [wrp, stp, qkp, ptp, rbp]

        def load_w8(Wsrc, w_t=None, t_only=None):
            # [128, hilo, c, d] fp8
            if w_t is None:
                w_t = wrp.tile([128, 2, NCH, D], f8, tag="W8", name="W8",
                               bufs=2)
            src = Wsrc.rearrange("t (c p) d -> p t c d", p=128)
            for t in ((0, 1) if t_only is None else (t_only,)):
                nc.sync.dma_start(out=w_t[:, t, 0:4, :], in_=src[:, t, 0:4, :])
                nc.scalar.dma_start(out=w_t[:, t, 4:8, :], in_=src[:, t, 4:8, :])
            return w_t

        def load_wo(Wsrc):
            w_t = wrp.tile([128, NCH * D], bf16, tag="Wor", name="Wor")
            d3 = w_t[:].rearrange("p (c d) -> p c d", d=D)
            s3 = Wsrc.rearrange("(c p) d -> p c d", p=128)
            nc.sync.dma_start(out=d3[:, 0:4, :], in_=s3[:, 0:4, :])
            nc.scalar.dma_start(out=d3[:, 4:8, :], in_=s3[:, 4:8, :])
            return w_t

        def dr_products(emit, w_t, xt):
            """12 DoubleRow matmuls accumulating hi*hi + hi*lo + lo*hi over
            4 chunk-pairs; emit(lhs_sel, rhs_sel, t, first, last)."""
            combos = ((0, 0), (0, 1), (1, 0))
            n = 0
            for t in range(4):
                for (wi, xi) in combos:
                    n += 1
                    emit(wi, xi, t, n == 1, n == 12)

        def proj_qk_twopass(w_t, b_t, s, outd, xt):
            # pass 1: hi*hi only (needs just the hi halves); pass 2: the two
            # cross products accumulated in PSUM then DVE-added into st
            sts = {}

            def pass1(jc, lh):
                ps = psum.tile([128, 512], f32, tag="pp", name="pp", bufs=2)
                for t in range(4):
                    nc.tensor.matmul(
                        ps[:],
                        lhsT=w_t[:, 0, 2 * t:2 * t + 2,
                                 128 * jc:128 * (jc + 1)],
                        rhs=xt[:, 0, 2 * t:2 * t + 2,
                               512 * lh:512 * (lh + 1)],
                        start=(t == 0), stop=(t == 3), perf_mode=DR)
                if lh == 0:
                    sts[jc] = stp.tile([128, L], bf16, tag="stq", name="st", bufs=9)
                nc.vector.tensor_scalar_add(
                    sts[jc][:, 512 * lh:512 * (lh + 1)], ps[:],
                    b_t[:, jc:jc + 1])

            def pass2(jc, lh):
                ps = psum.tile([128, 512], f32, tag="pp", name="pp", bufs=2)
                n = 0
                for t in range(4):
                    for (wi, xi) in ((0, 1), (1, 0)):
                        n += 1
                        nc.tensor.matmul(
                            ps[:],
                            lhsT=w_t[:, wi, 2 * t:2 * t + 2,
                                     128 * jc:128 * (jc + 1)],
                            rhs=xt[:, xi, 2 * t:2 * t + 2,
                                   512 * lh:512 * (lh + 1)],
                            start=(n == 1), stop=(n == 8), perf_mode=DR)
                sl = sts[jc][:, 512 * lh:512 * (lh + 1)]
                nc.vector.tensor_add(sl, sl, ps[:])
                if lh == 1:
                    nc.scalar.dma_start(
                        out=outd[s][128 * jc:128 * (jc + 1), :],
                        in_=sts[jc][:])

            return ([lambda jc=jc, lh=lh: pass1(jc, lh)
                     for jc in range(NCH) for lh in range(2)]
                    + [lambda jc=jc, lh=lh: pass2(jc, lh)
                       for jc in range(NCH) for lh in range(2)])

        def proj_qk_blocks(w_t, b_t, s, outd, xt):
            # out [j, L] = (32 x_s W)^T, one block per jc
            def block(jc):
                def run():
                    st = stp.tile([128, L], bf16, tag="stq", name="st", bufs=9)
                    for lh in range(2):
                        ps = psum.tile([128, 512], f32, tag="pp",
                                       name="pp", bufs=2)

                        def emit(wi, xi, t, first, last, lh=lh, ps=ps):
                            nc.tensor.matmul(
                                ps[:],
                                lhsT=w_t[:, wi, 2 * t:2 * t + 2,
                                         128 * jc:128 * (jc + 1)],
                                rhs=xt[:, xi, 2 * t:2 * t + 2,
                                       512 * lh:512 * (lh + 1)],
                                start=first, stop=last, perf_mode=DR)
                        dr_products(emit, w_t, xt)
                        nc.vector.tensor_scalar_add(
                            st[:, 512 * lh:512 * (lh + 1)], ps[:],
                            b_t[:, jc:jc + 1])
                    nc.scalar.dma_start(
                        out=outd[s][128 * jc:128 * (jc + 1), :], in_=st[:])
                return run
            return [block(jc) for jc in range(NCH)]

        def proj_v_blocks(w_t, s, xt):
            # out [L, j] with interleaved 32.0 columns, one block per lc
            def block(lc):
                def run():
                    for jh in range(2):
                        ps = psum.tile([128, 512], f32, tag="pp",
                                       name="pp", bufs=2)

                        def emit(wi, xi, t, first, last, jh=jh, ps=ps):
                            nc.tensor.matmul(
                                ps[:],
                                lhsT=xt[:, xi, 2 * t:2 * t + 2,
                                        128 * lc:128 * (lc + 1)],
                                rhs=w_t[:, wi, 2 * t:2 * t + 2,
                                        512 * jh:512 * (jh + 1)],
                                start=first, stop=last, perf_mode=DR)
                        dr_products(emit, w_t, xt)
                        vst = stp.tile([128, 8 * 65], bf16, tag="stv", name="vst")
                        r = vst[:].rearrange("p (h w) -> p h w", w=65)
                        q3 = ps[:].rearrange("p (h w) -> p h w", w=64)
                        nc.vector.tensor_copy(r[:, :, 0:64], q3)
                        nc.vector.tensor_copy(
                            r[:, :, 64:65].squeeze(2), v32c[:, 0:8])
                        nc.scalar.dma_start(
                            out=v_d[s][128 * lc:128 * (lc + 1),
                                       8 * 65 * jh:8 * 65 * (jh + 1)],
                            in_=vst[:])
                return run
            return [block(lc) for lc in range(NCH)]

        def proj_v_units(w_t, s, xt):
            # one unit per (lc, jh) half-block
            def unit(lc, jh):
                def run():
                    ps = psum.tile([128, 512], f32, tag="pp",
                                   name="pp", bufs=2)

                    def emit(wi, xi, t, first, last):
                        nc.tensor.matmul(
                            ps[:],
                            lhsT=xt[:, xi, 2 * t:2 * t + 2,
                                    128 * lc:128 * (lc + 1)],
                            rhs=w_t[:, wi, 2 * t:2 * t + 2,
                                    512 * jh:512 * (jh + 1)],
                            start=first, stop=last, perf_mode=DR)
                    dr_products(emit, w_t, xt)
                    vst = stp.tile([128, 8 * 65], bf16, tag="stv", name="vst")
                    r = vst[:].rearrange("p (h w) -> p h w", w=65)
                    q3 = ps[:].rearrange("p (h w) -> p h w", w=64)
                    nc.vector.tensor_copy(r[:, :, 0:64], q3)
                    nc.vector.tensor_copy(
                        r[:, :, 64:65].squeeze(2), v32c[:, 0:8])
                    nc.scalar.dma_start(
                        out=v_d[s][128 * lc:128 * (lc + 1),
                                   8 * 65 * jh:8 * 65 * (jh + 1)],
                        in_=vst[:])
                return run
            return [unit(lc, jh) for lc in range(NCH) for jh in range(2)]

        def attention_blocks(m, OT):
            sa, sb, sv = MHA_STREAMS[m]
            pend = {}
            pend2 = {}

            def finalize(h, oc):
                po, co = 64 * (h % 2), (h // 2) * L
                rb_r = rbp.tile([1, L], f32r, tag="rbr", name="rbr")
                with nc.allow_low_precision(reason="recip feeds f32r bcast"):
                    nc.vector.reciprocal(rb_r[0:1, :], oc[64:65, :])
                for qh in range(2):
                    rb_ps = psum.tile([64, 512], f32, tag="pp",
                                      name="rb_ps", bufs=2)
                    nc.tensor.matmul(
                        rb_ps[:], lhsT=ones_r[0:1, :],
                        rhs=rb_r[0:1, 512 * qh:512 * (qh + 1)],
                        start=True, stop=True)
                    nc.vector.tensor_mul(
                        OT[po:po + 64, co + 512 * qh:co + 512 * (qh + 1)],
                        oc[0:64, 512 * qh:512 * (qh + 1)], rb_ps[:])

            def step_pend(h, oc):
                if pend:
                    (h1, oc1), = pend.items()
                    finalize(h1, oc1)
                    pend.clear()
                if h is not None:
                    pend[h] = oc

            def head_units(h):
                """Chunk-granular thunks: [load+QK0, (QK1,PV0), ...,
                (QK7,PV6), (PV7,copies,finalize-prev)]."""
                st = {}

                def qk(c):
                    s_ps = psum.tile([128, L], f32, tag="scr", name="scr",
                                     bufs=2)
                    for qh in range(2):
                        nc.tensor.matmul(
                            s_ps[:, 512 * qh:512 * (qh + 1)],
                            lhsT=st["kcat"][:, 128 * c:128 * (c + 1)],
                            rhs=st["qcat"][:, 512 * qh:512 * (qh + 1)],
                            start=True, stop=True)
                    p_sb = ptp.tile([128, L], bf16, tag="p_sb", name="p_sb")
                    nc.scalar.activation(p_sb[:], s_ps[:], AF.Exp, scale=SCALE)
                    st[c] = p_sb

                def pv(c):
                    for qh in range(2):
                        nc.tensor.matmul(
                            st["o_ps"][qh][0:65, :],
                            lhsT=st["vext"][:, 65 * c:65 * (c + 1)],
                            rhs=st[c][:, 512 * qh:512 * (qh + 1)],
                            start=(c == 0), stop=(c == NCH - 1))
                    del st[c]

                def u_load():
                    qcat = qkp.tile([128, L], bf16, tag="qcat", name="qcat",
                                    bufs=6)
                    kcat = qkp.tile([128, L], bf16, tag="kcat", name="kcat",
                                    bufs=6)
                    nc.sync.dma_start(
                        out=qcat[0:64, :], in_=qT_d[sa][64 * h:64 * h + 64, :])
                    nc.sync.dma_start(
                        out=qcat[64:128, :], in_=qT_d[sb][64 * h:64 * h + 64, :])
                    nc.sync.dma_start(
                        out=kcat[0:64, :], in_=kT_d[sa][64 * h:64 * h + 64, :])
                    nc.sync.dma_start(
                        out=kcat[64:128, :], in_=kT_d[sb][64 * h:64 * h + 64, :])
                    vext = qkp.tile([128, NCH * 65], bf16, tag="vext",
                                    name="vext", bufs=4)
                    vsrc = v_d[sv].rearrange("(c p) w -> p c w", p=128)
                    nc.sync.dma_start(
                        out=vext[:].rearrange("p (c w) -> p c w", w=65),
                        in_=vsrc[:, :, 65 * h:65 * (h + 1)])
                    st["qcat"], st["kcat"], st["vext"] = qcat, kcat, vext
                    st["o_ps"] = [psum.tile([65, 512], f32, tag="ops",
                                            name="ops", bufs=2)
                                  for _ in range(2)]
                    qk(0)

                def u_mid(c):
                    qk(c)
                    pv(c - 1)

                def u_tail():
                    pv(NCH - 1)
                    oc = stp.tile([65, L], f32, tag="stoc", name="oc",
                                  bufs=6)
                    for qh in range(2):
                        nc.vector.tensor_copy(
                            oc[:, 512 * qh:512 * (qh + 1)],
                            st["o_ps"][qh][:])
                    step_pend(h, oc)

                return ([u_load] + [lambda c=c: u_mid(c)
                                    for c in range(1, NCH)] + [u_tail])

            def tail():
                step_pend(None, None)

            units = []
            for h in range(H):
                units += head_units(h)
            units.append(tail)
            return units

        def oproj_units(m, OT, wo_t):
            # one unit per (qc, dh) half-block; DMA issued on the dh=1 unit
            osts = {}

            def unit(qc, dh):
                def run():
                    if dh == 0:
                        osts[qc] = stp.tile([128, L], f32, tag="stoc",
                                            name="ost", bufs=6)
                    ost = osts[qc]
                    op_ps = psum.tile([128, 512], f32, tag="pp",
                                      name="pp", bufs=2)
                    for c in range(NCH):
                        nc.tensor.matmul(
                            op_ps[:],
                            lhsT=OT[:, L * c + 128 * qc:L * c + 128 * (qc + 1)],
                            rhs=wo_t[:, D * c + 512 * dh:D * c + 512 * (dh + 1)],
                            start=(c == 0), stop=(c == NCH - 1))
                    nc.vector.tensor_copy(
                        ost[:, 512 * dh:512 * (dh + 1)], op_ps[:])
                    if dh == 1:
                        nc.scalar.dma_start(
                            out=out[128 * qc:128 * (qc + 1), m, :], in_=ost[:])
                        del osts[qc]
                return run
            return [unit(qc, dh) for qc in range(NCH) for dh in range(2)]

        for _rep in range(repeat):
            # ---------------- emission schedule ----------------
            OTs = {}

            def mk_ot(m):
                OTs[m] = xts.tile([128, NCH * L], bf16, tag="xts", name="OT")

            wq_t = load_w8(Wq8, t_only=0)
            load_xt(1, t_only=0)
            load_w8(Wq8, w_t=wq_t, t_only=1)
            load_xt(1, t_only=1)
            load_xt(2)
            load_xt(0)
            for b in proj_qk_twopass(wq_t, bq_t, 1, qT_d, xt_tiles[1]):
                b()
            wk_t = load_w8(Wk8)
            for s in (2, 0):
                for b in proj_qk_blocks(wq_t, bq_t, s, qT_d, xt_tiles[s]):
                    b()
            for b in proj_qk_blocks(wk_t, bk_t, 1, kT_d, xt_tiles[1]):
                b()
            wv_t = load_w8(Wv8)
            for s in (2, 0):
                for b in proj_qk_blocks(wk_t, bk_t, s, kT_d, xt_tiles[s]):
                    b()

            for b in proj_v_blocks(wv_t, 0, xt_tiles[0]):
                b()

            # A0 || (v1, v2): xt1/xt2 and Wv still resident
            mk_ot(0)

            _interleave(attention_blocks(0, OTs[0]),
                        (proj_v_units(wv_t, 1, xt_tiles[1])
                         + proj_v_units(wv_t, 2, xt_tiles[2]), 0.25))

            # A1 || (load Wo, oproj 0)
            mk_ot(1)
            wo_state = {}

            def o0_blocks():
                blocks = []

                def loadwo():
                    wo_state["w"] = load_wo(Wo)
                blocks.append(loadwo)

                def get_units():
                    if "u0" not in wo_state:
                        wo_state["u0"] = oproj_units(0, OTs[0], wo_state["w"])
                    return wo_state["u0"]
                for u in range(16):
                    blocks.append(lambda u=u: get_units()[u]())
                return blocks

            _interleave(attention_blocks(1, OTs[1]), (o0_blocks(), 0.25))

            # A2 || oproj 1
            mk_ot(2)
            o1_units = oproj_units(1, OTs[1], wo_state["w"])
            _interleave(attention_blocks(2, OTs[2]),
                        ([lambda u=u: o1_units[u]() for u in range(16)], 0.25))

            for u in oproj_units(2, OTs[2], wo_state["w"]):
                u()

        for p in reversed(cstack):
            p.release()

    _split_excess_waits(nc, max_waits=1)
    return nc


def get_program():
    if "nc" not in _CACHE:
        _CACHE["nc"] = _build_program()
    return _CACHE["nc"]


def _split_fp8(a, axis=0):
    """a (f32) -> (hi, lo) fp8e4m3 stacked on `axis` with hi + lo ~= a."""
    import ml_dtypes

    hi = a.astype(ml_dtypes.float8_e4m3)
    lo = (a - hi.astype(np.float32)).astype(ml_dtypes.float8_e4m3)
    return np.ascontiguousarray(np.stack([hi, lo], axis=axis))


def kernel(x, Wq, bq, Wk, bk, Wv, bv, Wo, bo):
    import ml_dtypes
    from concourse.bass_utils import run_bass_kernel_spmd

    nc = get_program()
    x = np.ascontiguousarray(np.asarray(x, dtype=np.float32))
    Wq = np.asarray(Wq, dtype=np.float32)
    Wk = np.asarray(Wk, dtype=np.float32)
    Wv = np.asarray(Wv, dtype=np.float32)
    ws = {
        "Wq8": _split_fp8(WSCL * Wq),
        "Wk8": _split_fp8(WSCL * Wk),
        "Wv8": _split_fp8(WSCL * Wv),
        "Wo": np.ascontiguousarray(np.asarray(Wo, dtype=np.float32).astype(ml_dtypes.bfloat16)),
        "bq": WSCL * np.asarray(bq, dtype=np.float32),
        "bk": WSCL * np.asarray(bk, dtype=np.float32),
    }
    bv = np.asarray(bv, dtype=np.float64)
    bo = np.asarray(bo, dtype=np.float64)
    in_maps = [
        dict(ws, xT8=_split_fp8(x[b].transpose(1, 2, 0), axis=1))
        for b in range(N_CORES)
    ]
    res = run_bass_kernel_spmd(nc, in_maps, list(range(N_CORES)))
    outp = np.stack([res.results[b]["out"] for b in range(N_CORES)], axis=0)
    # bv and bo fold into a constant output row: softmax rows sum to 1, so
    # attention(v + bv) = attention(v) + bv, and (o + bv) @ Wo + bo adds
    # (bv @ Wo + bo) to every output row.
    corr = bv @ np.asarray(Wo, dtype=np.float64) + bo
    if np.any(corr):
        outp = (outp.astype(np.float64) + corr[None, None, None, :]).astype(
            np.float32)
    return outp
